# revision 2
# baseline (speedup 1.0000x reference)
"""Llama GQA attention (B=2, S=2048, H=4096, 32 q heads / 8 kv heads, HD=128)
on 8 Trainium2 NeuronCores — collective-free.

Sharding: DP=2 over batch x TP=4 over heads.
  core c: batch b = c // 4, tp rank r = c % 4
  - owns q heads [8r, 8r+8), kv heads [2r, 2r+2)
  - computes attention for its heads over its batch
  - o_proj ROW-sharded: each core multiplies its 1024 attn features by its
    1024 rows of wo, producing a PARTIAL full-width [4096, S] output.
    The host sums the 4 partials per batch (no on-device collectives —
    they hit a ~100x degraded execution mode on this terminal).

All dtype casts happen on the host (weights/x uploaded bf16 in panel
layout), so the device only does DMA + matmul + rope/softmax.

On-chip layout is fully "transposed" ([feature, token]):
  QT/KT: [d, t] (weight tiles stationary, X^T moving)
  V:     [t, d] (X^T tiles stationary, wv moving)
  S^T[k, q] = (KT tile).T @ QT          (contraction d on partitions)
  P^T = exp(scale * S^T)                (ScalarE, fp32 PSUM -> bf16 SBUF)
  attn^T[d, q] += (V tile).T @ P^T      (contraction k-tokens on partitions)
  denom[*, q] += ones128.T @ P^T        (col-sums replicated on partitions)
  out^T[oc, t] += (wo tile).T @ attn^T  (partial, contraction 1024 local d)
Causal masking: only lower-triangle k-tiles are computed; the diagonal
128x512 tiles use one of 4 static 0/1 masks (multiplied into P^T on GpSimd).
Softmax skips max-subtraction (scores are O(7), exp fits fp32 comfortably).

The attention QK matmuls are issued 2 iterations ahead of the PV/denom
matmuls so the PE never stalls on the ScalarE exp chain.
"""

import sys

for _p in ("/opt/trn_rl_repo",):
    if _p not in sys.path:
        sys.path.append(_p)

import numpy as np
import ml_dtypes

import concourse.bacc as bacc
import concourse.mybir as mybir
import concourse.tile as tile
from concourse.bass_utils import run_bass_kernel_spmd

F32 = mybir.dt.float32
BF16 = mybir.dt.bfloat16

B, S, H = 2, 2048, 4096
NH, NKV, HD = 32, 8, 128
N_CORES = 8
TP = 4

HL = NH // TP          # 8 local q heads
KVL = NKV // TP        # 2 local kv heads

TC = 512               # token chunk (= one attention q-block)
NCHUNK = S // TC       # 4
KT = H // 128          # 32 contraction tiles for the projections
SCALE = float(HD ** -0.5)
AHEAD = 2              # QK matmuls issued this many k-tiles early

LAST_RESULT = None
_BUILT = {}


def _build():
    nc = bacc.Bacc("TRN2", debug=False, num_devices=N_CORES)

    # all bf16, host-precast, panel layout [p, kt*128 + c] (lhsT tiles)
    xt_d = nc.dram_tensor("xt", [128, KT * S], BF16, kind="ExternalInput").ap()
    wqp_d = nc.dram_tensor("wqp", [HL, 128, KT * 128], BF16, kind="ExternalInput").ap()
    wkp_d = nc.dram_tensor("wkp", [KVL, 128, KT * 128], BF16, kind="ExternalInput").ap()
    wvp_d = nc.dram_tensor("wvp", [KVL, 128, KT * 128], BF16, kind="ExternalInput").ap()
    # wo rows [1024r, 1024(r+1)), all 4096 out cols: per oc-panel lhsT tiles
    wop_d = nc.dram_tensor("wop", [H // 128, 128, HL * 128], BF16, kind="ExternalInput").ap()
    cos_d = nc.dram_tensor("cos_t", [HD, S], F32, kind="ExternalInput").ap()
    sin_d = nc.dram_tensor("sin_t", [HD, S], F32, kind="ExternalInput").ap()
    mask_d = nc.dram_tensor("masks", [HD, 4 * TC], BF16, kind="ExternalInput").ap()
    ones_d = nc.dram_tensor("onesb", [128, 128], BF16, kind="ExternalInput").ap()
    out_d = nc.dram_tensor("out_t", [H, S], F32, kind="ExternalOutput").ap()

    with tile.TileContext(nc) as tc:
        with tc.tile_pool(name="sb", bufs=1) as sb, \
             tc.tile_pool(name="ps", bufs=1, space="PSUM") as ps:

            # ---- persistent tiles ----
            cos_sb = sb.tile([HD, S], F32)
            sin_sb = sb.tile([HD, S], F32)
            mask_sb = sb.tile([HD, 4 * TC], BF16)
            ones_sb = sb.tile([128, 128], BF16)
            ktb = sb.tile([128, KVL * S], BF16)      # roped K^T, [d, kv*S + t]
            vb = sb.tile([128, (S // 128) * KVL * 128], BF16)  # V, [t, tt*256 + d]
            nc.sync.dma_start(cos_sb[:], cos_d[:])
            nc.sync.dma_start(sin_sb[:], sin_d[:])
            nc.sync.dma_start(mask_sb[:], mask_d[:])
            nc.sync.dma_start(ones_sb[:], ones_d[:])

            KVCOLS = KVL * 128   # 256

            def get_panel(src_ap, idx, nbufs=2):
                """[128, KT*128] bf16 panel, one contiguous 1MB DMA."""
                wb = sb.tile([128, KT * 128], BF16, tag="wb", bufs=nbufs)
                nc.sync.dma_start(wb[:], src_ap[idx])
                return wb

            _XTB = {}

            def load_xt(c):
                """One 4MB DMA for the whole [4096, TC] bf16 X^T chunk."""
                _XTB[c] = sb.tile([128, KT * TC], BF16, tag="big", bufs=2,
                                  name=f"xtb{c}")
                t0 = c * TC
                nc.sync.dma_start(
                    _XTB[c].rearrange("p (kt t) -> p kt t", t=TC),
                    xt_d.rearrange("p (kt t) -> p kt t", t=S)[:, :, t0:t0 + TC])

            def rope(dst, pq, t0):
                """dst (bf16 [128, TC]) = rope of pq (fp32 PSUM [128, TC])."""
                qf = sb.tile([128, TC], F32, tag="qf", bufs=2)
                nc.scalar.copy(qf[:], pq[:])
                qs = sb.tile([128, TC], F32, tag="qs", bufs=2)
                nc.sync.dma_start(qs[0:64, :], qf[64:128, :])
                nc.sync.dma_start(qs[64:128, :], qf[0:64, :])
                nc.vector.tensor_tensor(
                    qf[:], qf[:], cos_sb[:, t0:t0 + TC], mybir.AluOpType.mult)
                nc.vector.tensor_tensor(
                    qs[:], qs[:], sin_sb[:, t0:t0 + TC], mybir.AluOpType.mult)
                nc.vector.tensor_tensor(dst, qf[:], qs[:], mybir.AluOpType.add)

            def proj(c):
                t0 = c * TC
                qtb = sb.tile([128, HL * TC], BF16, tag="qt", bufs=2, name="qtb")
                for h in range(HL):
                    wb = get_panel(wqp_d, h)
                    pq = ps.tile([128, TC], F32, tag="pj", bufs=2, name="pq")
                    for kt in range(KT):
                        nc.tensor.matmul(
                            pq[:], wb[:, kt * 128:(kt + 1) * 128],
                            _XTB[c][:, kt * TC:(kt + 1) * TC],
                            start=(kt == 0), stop=(kt == KT - 1))
                    rope(qtb[:, h * TC:(h + 1) * TC], pq, t0)
                for kv in range(KVL):
                    wb = get_panel(wkp_d, kv)
                    pk = ps.tile([128, TC], F32, tag="pj", bufs=2, name="pk")
                    for kt in range(KT):
                        nc.tensor.matmul(
                            pk[:], wb[:, kt * 128:(kt + 1) * 128],
                            _XTB[c][:, kt * TC:(kt + 1) * TC],
                            start=(kt == 0), stop=(kt == KT - 1))
                    rope(ktb[:, kv * S + t0:kv * S + t0 + TC], pk, t0)
                wv0 = get_panel(wvp_d, 0)
                wv1 = get_panel(wvp_d, 1)
                for tt_ in range(TC // 128):
                    # separate psum banks per half: start=True clears
                    # has_written for the WHOLE bank.
                    pv0 = ps.tile([128, 128], F32, tag="pj", bufs=2, name="pv0")
                    pv1 = ps.tile([128, 128], F32, tag="pj", bufs=2, name="pv1")
                    for kt in range(KT):
                        lx = _XTB[c][:, kt * TC + tt_ * 128:kt * TC + (tt_ + 1) * 128]
                        nc.tensor.matmul(
                            pv0[:], lx, wv0[:, kt * 128:(kt + 1) * 128],
                            start=(kt == 0), stop=(kt == KT - 1))
                        nc.tensor.matmul(
                            pv1[:], lx, wv1[:, kt * 128:(kt + 1) * 128],
                            start=(kt == 0), stop=(kt == KT - 1))
                    vt_idx = (t0 // 128) + tt_
                    nc.scalar.copy(
                        vb[:, vt_idx * KVCOLS:vt_idx * KVCOLS + 128], pv0[:])
                    nc.scalar.copy(
                        vb[:, vt_idx * KVCOLS + 128:(vt_idx + 1) * KVCOLS], pv1[:])
                return qtb

            def attention(c, qtb, attnb, half):
                """Attention for q block c into attnb[:, h*2TC + half*TC]."""
                nkt = 4 * c + 4
                for h in range(HL):
                    kv = h // (HL // KVL)
                    qsl = qtb[:, h * TC:(h + 1) * TC]
                    pa = ps.tile([128, TC], F32, tag="pa", bufs=2, name="pa")
                    pd = ps.tile([128, TC], F32, tag="den", bufs=1, name="pd")
                    sps_q = []

                    def qk(kt):
                        sps = ps.tile([128, TC], F32, tag="s", bufs=AHEAD + 1,
                                      name="sps")
                        nc.tensor.matmul(
                            sps[:],
                            ktb[:, kv * S + kt * 128:kv * S + (kt + 1) * 128],
                            qsl, start=True, stop=True)
                        sps_q.append(sps)

                    for kt in range(min(AHEAD, nkt)):
                        qk(kt)
                    for kt in range(nkt):
                        if kt + AHEAD < nkt:
                            qk(kt + AHEAD)
                        sps = sps_q[kt]
                        pt = sb.tile([128, TC], BF16, tag="pt", bufs=4, name="pt")
                        nc.scalar.activation(
                            pt[:], sps[:], mybir.ActivationFunctionType.Exp,
                            scale=SCALE)
                        j = kt - 4 * c
                        if j >= 0:
                            # on GpSimd: keeps the (busy) DVE out of the
                            # exp->mask->matmul critical chain
                            nc.gpsimd.tensor_tensor(
                                pt[:], pt[:], mask_sb[:, j * TC:(j + 1) * TC],
                                mybir.AluOpType.mult)
                        nc.tensor.matmul(
                            pa[:],
                            vb[:, kt * KVCOLS + kv * 128:kt * KVCOLS + (kv + 1) * 128],
                            pt[:], start=(kt == 0), stop=(kt == nkt - 1))
                        nc.tensor.matmul(
                            pd[:], ones_sb[:], pt[:],
                            start=(kt == 0), stop=(kt == nkt - 1))
                    rc = sb.tile([128, TC], F32, tag="rc", bufs=2, name="rc")
                    nc.vector.reciprocal(rc[:], pd[:])
                    dst = attnb[:, h * 2 * TC + half * TC:h * 2 * TC + (half + 1) * TC]
                    nc.vector.tensor_tensor(dst, pa[:], rc[:],
                                            mybir.AluOpType.mult)

            def outproj_pair(cpair, attnb):
                """Partial o_proj for chunks (2*cpair, 2*cpair+1)."""
                for o in range(H // 128):
                    wb = sb.tile([128, HL * 128], BF16, tag="wo", bufs=3)
                    nc.sync.dma_start(wb[:], wop_d[o])
                    for half in range(2):
                        po = ps.tile([128, TC], F32, tag="pj", bufs=2, name="po")
                        for h in range(HL):
                            nc.tensor.matmul(
                                po[:], wb[:, h * 128:(h + 1) * 128],
                                attnb[:, h * 2 * TC + half * TC:
                                      h * 2 * TC + (half + 1) * TC],
                                start=(h == 0), stop=(h == HL - 1))
                        ot = sb.tile([128, TC], F32, tag="ot", bufs=3)
                        nc.scalar.copy(ot[:], po[:])
                        t0 = (2 * cpair + half) * TC
                        nc.sync.dma_start(
                            out_d[o * 128:(o + 1) * 128, t0:t0 + TC], ot[:])

            # ---- chunk loop ----
            load_xt(0)
            attnb = None
            for c in range(NCHUNK):
                if c % 2 == 0:
                    attnb = sb.tile([128, HL * 2 * TC], BF16, tag="attn",
                                    bufs=2, name=f"attnb{c // 2}")
                qtb = proj(c)
                if c + 1 < NCHUNK:
                    load_xt(c + 1)
                attention(c, qtb, attnb, c % 2)
                if c % 2 == 1:
                    outproj_pair(c // 2, attnb)

    nc.compile()
    return nc


def _get_nc():
    if "nc" not in _BUILT:
        _BUILT["nc"] = _build()
    return _BUILT["nc"]


def _to_bf16(a):
    return np.ascontiguousarray(a.astype(ml_dtypes.bfloat16))


def _panelize(w):
    """[4096, ncols] -> per-128-col panels as lhsT tiles [n, 128, KT*128]."""
    ncol = w.shape[1] // 128
    # panel for col block j: [4096, 128] -> [KT, 128, 128] -> [128, KT*128]
    p = w.reshape(KT, 128, ncol, 128).transpose(2, 1, 0, 3).reshape(
        ncol, 128, KT * 128)
    return np.ascontiguousarray(p)


def kernel(hidden_states, cos, sin, wq, wk, wv, wo):
    global LAST_RESULT
    nc = _get_nc()

    hidden_states = np.asarray(hidden_states, dtype=np.float32)
    cos = np.asarray(cos, dtype=np.float32)
    sin = np.asarray(sin, dtype=np.float32)
    wq = np.asarray(wq, dtype=np.float32)
    wk = np.asarray(wk, dtype=np.float32)
    wv = np.asarray(wv, dtype=np.float32)
    wo = np.asarray(wo, dtype=np.float32)

    # host-side shard prep, all bf16 panel layouts
    xts = []
    for b in range(B):
        xt = np.ascontiguousarray(hidden_states[b].T)        # [H, S]
        xts.append(_to_bf16(xt).reshape(128 * KT, S).reshape(KT, 128, S)
                   .transpose(1, 0, 2).reshape(128, KT * S))
    xts = [np.ascontiguousarray(x) for x in xts]
    cts = [np.ascontiguousarray(cos[b].T) for b in range(B)]
    sin_eff = []
    for b in range(B):
        st = np.ascontiguousarray(sin[b].T)
        se = st.copy()
        se[0:64, :] *= -1.0
        sin_eff.append(se)

    masks = np.zeros((HD, 4 * TC), dtype=ml_dtypes.bfloat16)
    for j in range(4):
        m = (np.arange(HD)[:, None] + 128 * j) <= np.arange(TC)[None, :]
        masks[:, j * TC:(j + 1) * TC] = m.astype(ml_dtypes.bfloat16)
    onesb = np.ones((128, 128), dtype=ml_dtypes.bfloat16)

    wq_bf = _to_bf16(wq)
    wk_bf = _to_bf16(wk)
    wv_bf = _to_bf16(wv)
    wo_bf = _to_bf16(wo)

    in_maps = []
    for core in range(N_CORES):
        b, r = core // TP, core % TP
        # o_proj rows for this core: its 1024 attn features
        wo_rows = wo_bf[r * HL * 128:(r + 1) * HL * 128, :]   # [1024, 4096]
        # per oc-panel lhsT tiles: [32, 128, HL*128]
        wop = wo_rows.reshape(HL, 128, H // 128, 128).transpose(2, 1, 0, 3)
        wop = np.ascontiguousarray(wop.reshape(H // 128, 128, HL * 128))
        in_maps.append({
            "xt": xts[b],
            "cos_t": cts[b],
            "sin_t": sin_eff[b],
            "wqp": _panelize(wq_bf[:, r * HL * 128:(r + 1) * HL * 128]),
            "wkp": _panelize(wk_bf[:, r * KVL * 128:(r + 1) * KVL * 128]),
            "wvp": _panelize(wv_bf[:, r * KVL * 128:(r + 1) * KVL * 128]),
            "wop": wop,
            "masks": masks,
            "onesb": onesb,
        })

    res = run_bass_kernel_spmd(nc, in_maps, core_ids=list(range(N_CORES)))
    LAST_RESULT = res

    out = np.zeros((B, S, H), dtype=np.float32)
    for core in range(N_CORES):
        b = core // TP
        out[b] += res.results[core]["out_t"].T
    return out


# revision 3
# speedup vs baseline: 1.0322x; 1.0322x over previous
"""Llama GQA attention (B=2, S=2048, H=4096, 32 q heads / 8 kv heads, HD=128)
on 8 Trainium2 NeuronCores — collective-free.

Sharding: DP=2 over batch x TP=4 over heads.
  core c: batch b = c // 4, tp rank r = c % 4
  - owns q heads [8r, 8r+8), kv heads [2r, 2r+2)
  - computes attention for its heads over its batch
  - o_proj ROW-sharded: each core multiplies its 1024 attn features by its
    1024 rows of wo, producing a PARTIAL full-width [4096, S] output.
    The host sums the 4 partials per batch (no on-device collectives —
    they hit a ~100x degraded execution mode on this terminal).

All dtype casts happen on the host (weights/x uploaded bf16 in panel
layout), so the device only does DMA + matmul + rope/softmax.

On-chip layout is fully "transposed" ([feature, token]):
  QT/KT: [d, t] (weight tiles stationary, X^T moving)
  V:     [t, d] (X^T tiles stationary, wv moving)
  S^T[k, q] = (KT tile).T @ QT          (contraction d on partitions)
  P^T = exp(scale * S^T)                (ScalarE, fp32 PSUM -> bf16 SBUF)
  attn^T[d, q] += (V tile).T @ P^T      (contraction k-tokens on partitions)
  denom[*, q] += ones128.T @ P^T        (col-sums replicated on partitions)
  out^T[oc, t] += (wo tile).T @ attn^T  (partial, contraction 1024 local d)
Causal masking: only lower-triangle k-tiles are computed; the diagonal
128x512 tiles use one of 4 static 0/1 masks (multiplied into P^T on GpSimd).
Softmax skips max-subtraction (scores are O(7), exp fits fp32 comfortably).

The attention QK matmuls are issued 2 iterations ahead of the PV/denom
matmuls so the PE never stalls on the ScalarE exp chain.
"""

import sys

for _p in ("/opt/trn_rl_repo",):
    if _p not in sys.path:
        sys.path.append(_p)

import numpy as np
import ml_dtypes

import concourse.bacc as bacc
import concourse.mybir as mybir
import concourse.tile as tile
from concourse.bass_utils import run_bass_kernel_spmd

F32 = mybir.dt.float32
BF16 = mybir.dt.bfloat16

B, S, H = 2, 2048, 4096
NH, NKV, HD = 32, 8, 128
N_CORES = 8
TP = 4

HL = NH // TP          # 8 local q heads
KVL = NKV // TP        # 2 local kv heads

TC = 512               # token chunk (= one attention q-block)
NCHUNK = S // TC       # 4
KT = H // 128          # 32 contraction tiles for the projections
SCALE = float(HD ** -0.5)
AHEAD = 2              # QK matmuls issued this many k-tiles early

LAST_RESULT = None
_BUILT = {}


def _build():
    nc = bacc.Bacc("TRN2", debug=False, num_devices=N_CORES)

    # all bf16, host-precast, panel layout [p, kt*128 + c] (lhsT tiles)
    xt_d = nc.dram_tensor("xt", [128, KT * S], BF16, kind="ExternalInput").ap()
    wqp_d = nc.dram_tensor("wqp", [HL, 128, KT * 128], BF16, kind="ExternalInput").ap()
    wkp_d = nc.dram_tensor("wkp", [KVL, 128, KT * 128], BF16, kind="ExternalInput").ap()
    wvp_d = nc.dram_tensor("wvp", [KVL, 128, KT * 128], BF16, kind="ExternalInput").ap()
    # wo rows [1024r, 1024(r+1)), all 4096 out cols: per oc-panel lhsT tiles
    wop_d = nc.dram_tensor("wop", [H // 128, 128, HL * 128], BF16, kind="ExternalInput").ap()
    cos_d = nc.dram_tensor("cos_t", [HD, S], F32, kind="ExternalInput").ap()
    sin_d = nc.dram_tensor("sin_t", [HD, S], F32, kind="ExternalInput").ap()
    mask_d = nc.dram_tensor("masks", [HD, 4 * TC], BF16, kind="ExternalInput").ap()
    ones_d = nc.dram_tensor("onesb", [128, 128], BF16, kind="ExternalInput").ap()
    out_d = nc.dram_tensor("out_t", [H, S], F32, kind="ExternalOutput").ap()

    with tile.TileContext(nc) as tc:
        with tc.tile_pool(name="sb", bufs=1) as sb, \
             tc.tile_pool(name="ps", bufs=1, space="PSUM") as ps:

            # ---- persistent tiles ----
            cos_sb = sb.tile([HD, S], F32)
            sin_sb = sb.tile([HD, S], F32)
            mask_sb = sb.tile([HD, 4 * TC], BF16)
            ones_sb = sb.tile([128, 128], BF16)
            ktb = sb.tile([128, KVL * S], BF16)      # roped K^T, [d, kv*S + t]
            vb = sb.tile([128, (S // 128) * KVL * 128], BF16)  # V, [t, tt*256 + d]
            nc.sync.dma_start(cos_sb[:], cos_d[:])
            nc.sync.dma_start(sin_sb[:], sin_d[:])
            nc.sync.dma_start(mask_sb[:], mask_d[:])
            nc.sync.dma_start(ones_sb[:], ones_d[:])

            KVCOLS = KVL * 128   # 256

            def get_panel(src_ap, idx, nbufs=2):
                """[128, KT*128] bf16 panel, one contiguous 1MB DMA."""
                wb = sb.tile([128, KT * 128], BF16, tag="wb", bufs=nbufs)
                nc.sync.dma_start(wb[:], src_ap[idx])
                return wb

            _XTB = {}

            def load_xt(c, strips=1):
                """[4096, TC] bf16 X^T chunk; `strips` parallel DMAs so the
                first k-tiles land early (chunk 0 gates the cold start)."""
                _XTB[c] = sb.tile([128, KT * TC], BF16, tag="big", bufs=2,
                                  name=f"xtb{c}")
                t0 = c * TC
                dst = _XTB[c].rearrange("p (kt t) -> p kt t", t=TC)
                src = xt_d.rearrange("p (kt t) -> p kt t", t=S)[:, :, t0:t0 + TC]
                kper = KT // strips
                for s_ in range(strips):
                    nc.sync.dma_start(
                        dst[:, s_ * kper:(s_ + 1) * kper],
                        src[:, s_ * kper:(s_ + 1) * kper])

            def rope(dst, pq, t0):
                """dst (bf16 [128, TC]) = rope of pq (fp32 PSUM [128, TC])."""
                qf = sb.tile([128, TC], F32, tag="qf", bufs=2)
                nc.scalar.copy(qf[:], pq[:])
                qs = sb.tile([128, TC], F32, tag="qs", bufs=2)
                nc.sync.dma_start(qs[0:64, :], qf[64:128, :])
                nc.sync.dma_start(qs[64:128, :], qf[0:64, :])
                nc.vector.tensor_tensor(
                    qf[:], qf[:], cos_sb[:, t0:t0 + TC], mybir.AluOpType.mult)
                nc.vector.tensor_tensor(
                    qs[:], qs[:], sin_sb[:, t0:t0 + TC], mybir.AluOpType.mult)
                nc.vector.tensor_tensor(dst, qf[:], qs[:], mybir.AluOpType.add)

            def proj(c):
                t0 = c * TC
                qtb = sb.tile([128, HL * TC], BF16, tag="qt", bufs=2, name="qtb")
                for h in range(HL):
                    wb = get_panel(wqp_d, h)
                    pq = ps.tile([128, TC], F32, tag="pj", bufs=2, name="pq")
                    for kt in range(KT):
                        nc.tensor.matmul(
                            pq[:], wb[:, kt * 128:(kt + 1) * 128],
                            _XTB[c][:, kt * TC:(kt + 1) * TC],
                            start=(kt == 0), stop=(kt == KT - 1))
                    rope(qtb[:, h * TC:(h + 1) * TC], pq, t0)
                for kv in range(KVL):
                    wb = get_panel(wkp_d, kv)
                    pk = ps.tile([128, TC], F32, tag="pj", bufs=2, name="pk")
                    for kt in range(KT):
                        nc.tensor.matmul(
                            pk[:], wb[:, kt * 128:(kt + 1) * 128],
                            _XTB[c][:, kt * TC:(kt + 1) * TC],
                            start=(kt == 0), stop=(kt == KT - 1))
                    rope(ktb[:, kv * S + t0:kv * S + t0 + TC], pk, t0)
                wv0 = get_panel(wvp_d, 0)
                wv1 = get_panel(wvp_d, 1)
                for tt_ in range(TC // 128):
                    # separate psum banks per half: start=True clears
                    # has_written for the WHOLE bank.
                    pv0 = ps.tile([128, 128], F32, tag="pj", bufs=2, name="pv0")
                    pv1 = ps.tile([128, 128], F32, tag="pj", bufs=2, name="pv1")
                    for kt in range(KT):
                        lx = _XTB[c][:, kt * TC + tt_ * 128:kt * TC + (tt_ + 1) * 128]
                        nc.tensor.matmul(
                            pv0[:], lx, wv0[:, kt * 128:(kt + 1) * 128],
                            start=(kt == 0), stop=(kt == KT - 1))
                        nc.tensor.matmul(
                            pv1[:], lx, wv1[:, kt * 128:(kt + 1) * 128],
                            start=(kt == 0), stop=(kt == KT - 1))
                    vt_idx = (t0 // 128) + tt_
                    nc.scalar.copy(
                        vb[:, vt_idx * KVCOLS:vt_idx * KVCOLS + 128], pv0[:])
                    nc.scalar.copy(
                        vb[:, vt_idx * KVCOLS + 128:(vt_idx + 1) * KVCOLS], pv1[:])
                return qtb

            def attention(c, qtb, attnb, half):
                """Attention for q block c into attnb[:, h*2TC + half*TC]."""
                nkt = 4 * c + 4
                for h in range(HL):
                    kv = h // (HL // KVL)
                    qsl = qtb[:, h * TC:(h + 1) * TC]
                    pa = ps.tile([128, TC], F32, tag="pa", bufs=2, name="pa")
                    pd = ps.tile([128, TC], F32, tag="den", bufs=1, name="pd")
                    # plain (below-diagonal) k-tiles first, diagonal (masked)
                    # tiles last, so the mask's extra chain latency is hidden
                    # behind an already-deep pipeline
                    order = list(range(4 * c)) + list(range(4 * c, nkt))
                    sps_q = {}

                    def qk(pos):
                        kt = order[pos]
                        sps = ps.tile([128, TC], F32, tag="s", bufs=AHEAD + 1,
                                      name="sps")
                        nc.tensor.matmul(
                            sps[:],
                            ktb[:, kv * S + kt * 128:kv * S + (kt + 1) * 128],
                            qsl, start=True, stop=True)
                        sps_q[pos] = sps

                    for pos in range(min(AHEAD, nkt)):
                        qk(pos)
                    for pos in range(nkt):
                        if pos + AHEAD < nkt:
                            qk(pos + AHEAD)
                        kt = order[pos]
                        sps = sps_q.pop(pos)
                        pt = sb.tile([128, TC], BF16, tag="pt", bufs=4, name="pt")
                        nc.scalar.activation(
                            pt[:], sps[:], mybir.ActivationFunctionType.Exp,
                            scale=SCALE)
                        j = kt - 4 * c
                        if j >= 0:
                            # DVE, not GpSimd: ~4x lower latency in the
                            # exp->mask->matmul chain at head starts
                            nc.vector.tensor_tensor(
                                pt[:], pt[:], mask_sb[:, j * TC:(j + 1) * TC],
                                mybir.AluOpType.mult)
                        nc.tensor.matmul(
                            pa[:],
                            vb[:, kt * KVCOLS + kv * 128:kt * KVCOLS + (kv + 1) * 128],
                            pt[:], start=(pos == 0), stop=(pos == nkt - 1))
                        nc.tensor.matmul(
                            pd[:], ones_sb[:], pt[:],
                            start=(pos == 0), stop=(pos == nkt - 1))
                    rc = sb.tile([128, TC], F32, tag="rc", bufs=2, name="rc")
                    nc.vector.reciprocal(rc[:], pd[:])
                    dst = attnb[:, h * 2 * TC + half * TC:h * 2 * TC + (half + 1) * TC]
                    nc.vector.tensor_tensor(dst, pa[:], rc[:],
                                            mybir.AluOpType.mult)

            def outproj_pair(cpair, attnb):
                """Partial o_proj for chunks (2*cpair, 2*cpair+1)."""
                for o in range(H // 128):
                    wb = sb.tile([128, HL * 128], BF16, tag="wo", bufs=3)
                    nc.sync.dma_start(wb[:], wop_d[o])
                    for half in range(2):
                        po = ps.tile([128, TC], F32, tag="pj", bufs=2, name="po")
                        for h in range(HL):
                            nc.tensor.matmul(
                                po[:], wb[:, h * 128:(h + 1) * 128],
                                attnb[:, h * 2 * TC + half * TC:
                                      h * 2 * TC + (half + 1) * TC],
                                start=(h == 0), stop=(h == HL - 1))
                        ot = sb.tile([128, TC], F32, tag="ot", bufs=3)
                        nc.scalar.copy(ot[:], po[:])
                        t0 = (2 * cpair + half) * TC
                        nc.sync.dma_start(
                            out_d[o * 128:(o + 1) * 128, t0:t0 + TC], ot[:])

            # ---- chunk loop ----
            load_xt(0, strips=4)
            attnb = None
            for c in range(NCHUNK):
                if c % 2 == 0:
                    attnb = sb.tile([128, HL * 2 * TC], BF16, tag="attn",
                                    bufs=2, name=f"attnb{c // 2}")
                qtb = proj(c)
                if c + 1 < NCHUNK:
                    load_xt(c + 1)
                attention(c, qtb, attnb, c % 2)
                if c % 2 == 1:
                    outproj_pair(c // 2, attnb)

    nc.compile()
    return nc


def _get_nc():
    if "nc" not in _BUILT:
        _BUILT["nc"] = _build()
    return _BUILT["nc"]


def _to_bf16(a):
    return np.ascontiguousarray(a.astype(ml_dtypes.bfloat16))


def _panelize(w):
    """[4096, ncols] -> per-128-col panels as lhsT tiles [n, 128, KT*128]."""
    ncol = w.shape[1] // 128
    # panel for col block j: [4096, 128] -> [KT, 128, 128] -> [128, KT*128]
    p = w.reshape(KT, 128, ncol, 128).transpose(2, 1, 0, 3).reshape(
        ncol, 128, KT * 128)
    return np.ascontiguousarray(p)


def kernel(hidden_states, cos, sin, wq, wk, wv, wo):
    global LAST_RESULT
    nc = _get_nc()

    hidden_states = np.asarray(hidden_states, dtype=np.float32)
    cos = np.asarray(cos, dtype=np.float32)
    sin = np.asarray(sin, dtype=np.float32)
    wq = np.asarray(wq, dtype=np.float32)
    wk = np.asarray(wk, dtype=np.float32)
    wv = np.asarray(wv, dtype=np.float32)
    wo = np.asarray(wo, dtype=np.float32)

    # host-side shard prep, all bf16 panel layouts
    xts = []
    for b in range(B):
        xt = np.ascontiguousarray(hidden_states[b].T)        # [H, S]
        xts.append(_to_bf16(xt).reshape(128 * KT, S).reshape(KT, 128, S)
                   .transpose(1, 0, 2).reshape(128, KT * S))
    xts = [np.ascontiguousarray(x) for x in xts]
    cts = [np.ascontiguousarray(cos[b].T) for b in range(B)]
    sin_eff = []
    for b in range(B):
        st = np.ascontiguousarray(sin[b].T)
        se = st.copy()
        se[0:64, :] *= -1.0
        sin_eff.append(se)

    masks = np.zeros((HD, 4 * TC), dtype=ml_dtypes.bfloat16)
    for j in range(4):
        m = (np.arange(HD)[:, None] + 128 * j) <= np.arange(TC)[None, :]
        masks[:, j * TC:(j + 1) * TC] = m.astype(ml_dtypes.bfloat16)
    onesb = np.ones((128, 128), dtype=ml_dtypes.bfloat16)

    wq_bf = _to_bf16(wq)
    wk_bf = _to_bf16(wk)
    wv_bf = _to_bf16(wv)
    wo_bf = _to_bf16(wo)

    in_maps = []
    for core in range(N_CORES):
        b, r = core // TP, core % TP
        # o_proj rows for this core: its 1024 attn features
        wo_rows = wo_bf[r * HL * 128:(r + 1) * HL * 128, :]   # [1024, 4096]
        # per oc-panel lhsT tiles: [32, 128, HL*128]
        wop = wo_rows.reshape(HL, 128, H // 128, 128).transpose(2, 1, 0, 3)
        wop = np.ascontiguousarray(wop.reshape(H // 128, 128, HL * 128))
        in_maps.append({
            "xt": xts[b],
            "cos_t": cts[b],
            "sin_t": sin_eff[b],
            "wqp": _panelize(wq_bf[:, r * HL * 128:(r + 1) * HL * 128]),
            "wkp": _panelize(wk_bf[:, r * KVL * 128:(r + 1) * KVL * 128]),
            "wvp": _panelize(wv_bf[:, r * KVL * 128:(r + 1) * KVL * 128]),
            "wop": wop,
            "masks": masks,
            "onesb": onesb,
        })

    res = run_bass_kernel_spmd(nc, in_maps, core_ids=list(range(N_CORES)))
    LAST_RESULT = res

    out = np.zeros((B, S, H), dtype=np.float32)
    for core in range(N_CORES):
        b = core // TP
        out[b] += res.results[core]["out_t"].T
    return out


# revision 5
# speedup vs baseline: 1.0835x; 1.0496x over previous
"""Llama GQA attention (B=2, S=2048, H=4096, 32 q heads / 8 kv heads, HD=128)
on 8 Trainium2 NeuronCores — collective-free.

Sharding: DP=2 over batch x TP=4 over heads.
  core c: batch b = c // 4, tp rank r = c % 4
  - owns q heads [8r, 8r+8), kv heads [2r, 2r+2)
  - computes attention for its heads over its batch
  - o_proj ROW-sharded: each core multiplies its 1024 attn features by its
    1024 rows of wo, producing a PARTIAL full-width [4096, S] output.
    The host sums the 4 partials per batch (no on-device collectives —
    they hit a ~100x degraded execution mode on this terminal).

All dtype casts happen on the host (weights/x uploaded bf16 in panel
layout), so the device only does DMA + matmul + rope/softmax.

On-chip layout is fully "transposed" ([feature, token]):
  QT/KT: [d, t] (weight tiles stationary, X^T moving)
  V:     [t, d] (X^T tiles stationary, wv moving)
  S^T[k, q] = (KT tile).T @ QT          (contraction d on partitions)
  P^T = exp(scale * S^T)                (ScalarE, fp32 PSUM -> bf16 SBUF)
  attn^T[d, q] += (V tile).T @ P^T      (contraction k-tokens on partitions)
  denom[*, q] += ones128.T @ P^T        (col-sums replicated on partitions)
  out^T[oc, t] += (wo tile).T @ attn^T  (partial, contraction 1024 local d)
Causal masking: only lower-triangle k-tiles are computed; the diagonal
128x512 tiles use one of 4 static 0/1 masks (multiplied into P^T on GpSimd).
Softmax skips max-subtraction (scores are O(7), exp fits fp32 comfortably).

The attention QK matmuls are issued 2 iterations ahead of the PV/denom
matmuls so the PE never stalls on the ScalarE exp chain.
"""

import sys

for _p in ("/opt/trn_rl_repo",):
    if _p not in sys.path:
        sys.path.append(_p)

import numpy as np
import ml_dtypes

import concourse.bacc as bacc
import concourse.mybir as mybir
import concourse.tile as tile
from concourse.bass_utils import run_bass_kernel_spmd

F32 = mybir.dt.float32
BF16 = mybir.dt.bfloat16

B, S, H = 2, 2048, 4096
NH, NKV, HD = 32, 8, 128
N_CORES = 8
TP = 4

HL = NH // TP          # 8 local q heads
KVL = NKV // TP        # 2 local kv heads

TC = 512               # token chunk (= one attention q-block)
NCHUNK = S // TC       # 4
KT = H // 128          # 32 contraction tiles for the projections
SCALE = float(HD ** -0.5)
AHEAD = 3              # QK matmuls issued this many k-tiles early

LAST_RESULT = None
_BUILT = {}


def _build():
    nc = bacc.Bacc("TRN2", debug=False, num_devices=N_CORES)

    # all bf16, host-precast, panel layout [p, kt*128 + c] (lhsT tiles)
    xt_d = nc.dram_tensor("xt", [128, KT * S], BF16, kind="ExternalInput").ap()
    wqp_d = nc.dram_tensor("wqp", [HL, 128, KT * 128], BF16, kind="ExternalInput").ap()
    wkp_d = nc.dram_tensor("wkp", [KVL, 128, KT * 128], BF16, kind="ExternalInput").ap()
    wvp_d = nc.dram_tensor("wvp", [KVL, 128, KT * 128], BF16, kind="ExternalInput").ap()
    # wo rows [1024r, 1024(r+1)), all 4096 out cols: per oc-panel lhsT tiles
    wop_d = nc.dram_tensor("wop", [H // 128, 128, HL * 128], BF16, kind="ExternalInput").ap()
    cos_d = nc.dram_tensor("cos_t", [HD, S], F32, kind="ExternalInput").ap()
    sin_d = nc.dram_tensor("sin_t", [HD, S], F32, kind="ExternalInput").ap()
    mask_d = nc.dram_tensor("masks", [HD, 4 * TC], BF16, kind="ExternalInput").ap()
    ones_d = nc.dram_tensor("onesb", [128, 128], BF16, kind="ExternalInput").ap()
    out_d = nc.dram_tensor("out_t", [H, S], F32, kind="ExternalOutput").ap()

    with tile.TileContext(nc) as tc:
        with tc.tile_pool(name="sb", bufs=1) as sb, \
             tc.tile_pool(name="ps", bufs=1, space="PSUM") as ps:

            # ---- persistent tiles ----
            cos_sb = sb.tile([HD, S], F32)
            sin_sb = sb.tile([HD, S], F32)
            mask_sb = sb.tile([HD, 4 * TC], BF16)
            ones_sb = sb.tile([128, 128], BF16)
            ktb = sb.tile([128, KVL * S], BF16)      # roped K^T, [d, kv*S + t]
            vb = sb.tile([128, (S // 128) * KVL * 128], BF16)  # V, [t, tt*256 + d]

            def load_persistent():
                # emitted AFTER the chunk-0 x strips so they don't delay
                # the first projection matmuls; cos/sin only gate head 0's
                # rope, well after the strips land
                nc.scalar.dma_start(cos_sb[:], cos_d[:])
                nc.scalar.dma_start(sin_sb[:], sin_d[:])
                nc.scalar.dma_start(mask_sb[:], mask_d[:])
                nc.scalar.dma_start(ones_sb[:], ones_d[:])

            KVCOLS = KVL * 128   # 256

            def get_panel(src_ap, idx, nbufs=3):
                """[128, KT*128] bf16 panel, one contiguous 1MB DMA."""
                wb = sb.tile([128, KT * 128], BF16, tag="wb", bufs=nbufs)
                nc.sync.dma_start(wb[:], src_ap[idx])
                return wb

            _XTB = {}

            def load_xt(c, strips=1):
                """[4096, TC] bf16 X^T chunk; `strips` parallel DMAs so the
                first k-tiles land early (chunk 0 gates the cold start)."""
                _XTB[c] = sb.tile([128, KT * TC], BF16, tag="big", bufs=2,
                                  name=f"xtb{c}")
                t0 = c * TC
                dst = _XTB[c].rearrange("p (kt t) -> p kt t", t=TC)
                src = xt_d.rearrange("p (kt t) -> p kt t", t=S)[:, :, t0:t0 + TC]
                kper = KT // strips
                for s_ in range(strips):
                    # ACT's HWDGE ring, so x loads don't serialize behind
                    # the weight-panel DMAs on the sync ring
                    nc.scalar.dma_start(
                        dst[:, s_ * kper:(s_ + 1) * kper],
                        src[:, s_ * kper:(s_ + 1) * kper])

            def rope(dst, pq, t0):
                """dst (bf16 [128, TC]) = rope of pq (fp32 PSUM [128, TC])."""
                qf = sb.tile([128, TC], F32, tag="qf", bufs=2)
                nc.scalar.copy(qf[:], pq[:])
                qs = sb.tile([128, TC], F32, tag="qs", bufs=2)
                nc.sync.dma_start(qs[0:64, :], qf[64:128, :])
                nc.sync.dma_start(qs[64:128, :], qf[0:64, :])
                nc.vector.tensor_tensor(
                    qf[:], qf[:], cos_sb[:, t0:t0 + TC], mybir.AluOpType.mult)
                nc.vector.tensor_tensor(
                    qs[:], qs[:], sin_sb[:, t0:t0 + TC], mybir.AluOpType.mult)
                nc.vector.tensor_tensor(dst, qf[:], qs[:], mybir.AluOpType.add)

            def proj(c):
                t0 = c * TC
                qtb = sb.tile([128, HL * TC], BF16, tag="qt", bufs=2, name="qtb")
                for h in range(HL):
                    wb = get_panel(wqp_d, h)
                    pq = ps.tile([128, TC], F32, tag="pj", bufs=2, name="pq")
                    for kt in range(KT):
                        nc.tensor.matmul(
                            pq[:], wb[:, kt * 128:(kt + 1) * 128],
                            _XTB[c][:, kt * TC:(kt + 1) * TC],
                            start=(kt == 0), stop=(kt == KT - 1))
                    rope(qtb[:, h * TC:(h + 1) * TC], pq, t0)
                for kv in range(KVL):
                    wb = get_panel(wkp_d, kv)
                    pk = ps.tile([128, TC], F32, tag="pj", bufs=2, name="pk")
                    for kt in range(KT):
                        nc.tensor.matmul(
                            pk[:], wb[:, kt * 128:(kt + 1) * 128],
                            _XTB[c][:, kt * TC:(kt + 1) * TC],
                            start=(kt == 0), stop=(kt == KT - 1))
                    rope(ktb[:, kv * S + t0:kv * S + t0 + TC], pk, t0)
                wv0 = get_panel(wvp_d, 0)
                wv1 = get_panel(wvp_d, 1)
                for tt_ in range(TC // 128):
                    # separate psum banks per half: start=True clears
                    # has_written for the WHOLE bank.
                    pv0 = ps.tile([128, 128], F32, tag="pj", bufs=2, name="pv0")
                    pv1 = ps.tile([128, 128], F32, tag="pj", bufs=2, name="pv1")
                    for kt in range(KT):
                        lx = _XTB[c][:, kt * TC + tt_ * 128:kt * TC + (tt_ + 1) * 128]
                        nc.tensor.matmul(
                            pv0[:], lx, wv0[:, kt * 128:(kt + 1) * 128],
                            start=(kt == 0), stop=(kt == KT - 1))
                        nc.tensor.matmul(
                            pv1[:], lx, wv1[:, kt * 128:(kt + 1) * 128],
                            start=(kt == 0), stop=(kt == KT - 1))
                    vt_idx = (t0 // 128) + tt_
                    nc.scalar.copy(
                        vb[:, vt_idx * KVCOLS:vt_idx * KVCOLS + 128], pv0[:])
                    nc.scalar.copy(
                        vb[:, vt_idx * KVCOLS + 128:(vt_idx + 1) * KVCOLS], pv1[:])
                return qtb

            def attention(c, qtb, attnb, half):
                """Attention for q block c into attnb[:, h*2TC + half*TC]."""
                nkt = 4 * c + 4
                # unnormalized numerators/denominators, copied out per head
                # so the pa/pd PSUM banks free ~0.5us after the last PV
                # instead of after the 3.4us reciprocal chain; normalize
                # runs as a batch after the head loop, off the PE path
                pab = sb.tile([128, HL * TC], BF16, tag="pab", bufs=1,
                              name="pab")
                denb = sb.tile([128, HL * TC], BF16, tag="denb", bufs=1,
                               name="denb")
                for h in range(HL):
                    kv = h // (HL // KVL)
                    qsl = qtb[:, h * TC:(h + 1) * TC]
                    pa = ps.tile([128, TC], F32, tag="pa", bufs=1, name="pa")
                    pd = ps.tile([128, TC], F32, tag="den", bufs=1, name="pd")
                    # k-tiles processed in PAIRS: the two QK matmuls land in
                    # the two banks of one [128, 2*TC] PSUM tile, and ONE
                    # ACTIVATE exps both — cutting ACT's 352-cycle per-
                    # instruction overhead in half (ACT is the pacing engine
                    # of the attention phase otherwise). Diagonal (masked)
                    # pairs are naturally last; their mask slices are
                    # contiguous in mask_sb.
                    npairs = nkt // 2
                    sps_q = {}

                    def qk_pair(p):
                        sps = ps.tile([128, 2 * TC], F32, tag="s", bufs=2,
                                      name="sps")
                        for h_ in range(2):
                            kt = 2 * p + h_
                            nc.tensor.matmul(
                                sps[:, h_ * TC:(h_ + 1) * TC],
                                ktb[:, kv * S + kt * 128:kv * S + (kt + 1) * 128],
                                qsl, start=True, stop=True)
                        sps_q[p] = sps

                    for p0 in range(min(2, npairs)):
                        qk_pair(p0)
                    for p in range(npairs):
                        if p + 2 < npairs:
                            qk_pair(p + 2)
                        sps = sps_q.pop(p)
                        pt = sb.tile([128, 2 * TC], BF16, tag="pt", bufs=3,
                                     name="pt")
                        nc.scalar.activation(
                            pt[:], sps[:], mybir.ActivationFunctionType.Exp,
                            scale=SCALE)
                        j = 2 * p - 4 * c
                        if j >= 0:
                            # DVE, not GpSimd: ~4x lower latency in the
                            # exp->mask->matmul chain
                            nc.vector.tensor_tensor(
                                pt[:], pt[:], mask_sb[:, j * TC:(j + 2) * TC],
                                mybir.AluOpType.mult)
                        for h_ in range(2):
                            kt = 2 * p + h_
                            ptsl = pt[:, h_ * TC:(h_ + 1) * TC]
                            nc.tensor.matmul(
                                pa[:],
                                vb[:, kt * KVCOLS + kv * 128:kt * KVCOLS + (kv + 1) * 128],
                                ptsl, start=(kt == 0), stop=(kt == nkt - 1))
                            nc.tensor.matmul(
                                pd[:], ones_sb[:], ptsl,
                                start=(kt == 0), stop=(kt == nkt - 1))
                    nc.vector.tensor_copy(pab[:, h * TC:(h + 1) * TC], pa[:])
                    nc.vector.tensor_copy(denb[:, h * TC:(h + 1) * TC], pd[:])
                for h in range(HL):
                    rc = sb.tile([128, TC], F32, tag="rc", bufs=2, name="rc")
                    nc.vector.reciprocal(rc[:], denb[:, h * TC:(h + 1) * TC])
                    dst = attnb[:, h * 2 * TC + half * TC:h * 2 * TC + (half + 1) * TC]
                    nc.vector.tensor_tensor(dst, pab[:, h * TC:(h + 1) * TC],
                                            rc[:], mybir.AluOpType.mult)

            def outproj_pair(cpair, attnb):
                """Partial o_proj for chunks (2*cpair, 2*cpair+1)."""
                for o in range(H // 128):
                    wb = sb.tile([128, HL * 128], BF16, tag="wo", bufs=3)
                    nc.sync.dma_start(wb[:], wop_d[o])
                    for half in range(2):
                        po = ps.tile([128, TC], F32, tag="pj", bufs=2, name="po")
                        for h in range(HL):
                            nc.tensor.matmul(
                                po[:], wb[:, h * 128:(h + 1) * 128],
                                attnb[:, h * 2 * TC + half * TC:
                                      h * 2 * TC + (half + 1) * TC],
                                start=(h == 0), stop=(h == HL - 1))
                        ot = sb.tile([128, TC], F32, tag="ot", bufs=3)
                        nc.scalar.copy(ot[:], po[:])
                        t0 = (2 * cpair + half) * TC
                        nc.sync.dma_start(
                            out_d[o * 128:(o + 1) * 128, t0:t0 + TC], ot[:])

            # ---- chunk loop ----
            load_xt(0, strips=8)
            load_persistent()
            attnb = None
            for c in range(NCHUNK):
                if c % 2 == 0:
                    attnb = sb.tile([128, HL * 2 * TC], BF16, tag="attn",
                                    bufs=1, name=f"attnb{c // 2}")
                qtb = proj(c)
                if c + 1 < NCHUNK:
                    load_xt(c + 1)
                attention(c, qtb, attnb, c % 2)
                if c % 2 == 1:
                    outproj_pair(c // 2, attnb)

    nc.compile()
    return nc


def _get_nc():
    if "nc" not in _BUILT:
        _BUILT["nc"] = _build()
    return _BUILT["nc"]


def _to_bf16(a):
    return np.ascontiguousarray(a.astype(ml_dtypes.bfloat16))


def _panelize(w):
    """[4096, ncols] -> per-128-col panels as lhsT tiles [n, 128, KT*128]."""
    ncol = w.shape[1] // 128
    # panel for col block j: [4096, 128] -> [KT, 128, 128] -> [128, KT*128]
    p = w.reshape(KT, 128, ncol, 128).transpose(2, 1, 0, 3).reshape(
        ncol, 128, KT * 128)
    return np.ascontiguousarray(p)


def kernel(hidden_states, cos, sin, wq, wk, wv, wo):
    global LAST_RESULT
    nc = _get_nc()

    hidden_states = np.asarray(hidden_states, dtype=np.float32)
    cos = np.asarray(cos, dtype=np.float32)
    sin = np.asarray(sin, dtype=np.float32)
    wq = np.asarray(wq, dtype=np.float32)
    wk = np.asarray(wk, dtype=np.float32)
    wv = np.asarray(wv, dtype=np.float32)
    wo = np.asarray(wo, dtype=np.float32)

    # host-side shard prep, all bf16 panel layouts
    xts = []
    for b in range(B):
        xt = np.ascontiguousarray(hidden_states[b].T)        # [H, S]
        xts.append(_to_bf16(xt).reshape(128 * KT, S).reshape(KT, 128, S)
                   .transpose(1, 0, 2).reshape(128, KT * S))
    xts = [np.ascontiguousarray(x) for x in xts]
    cts = [np.ascontiguousarray(cos[b].T) for b in range(B)]
    sin_eff = []
    for b in range(B):
        st = np.ascontiguousarray(sin[b].T)
        se = st.copy()
        se[0:64, :] *= -1.0
        sin_eff.append(se)

    masks = np.zeros((HD, 4 * TC), dtype=ml_dtypes.bfloat16)
    for j in range(4):
        m = (np.arange(HD)[:, None] + 128 * j) <= np.arange(TC)[None, :]
        masks[:, j * TC:(j + 1) * TC] = m.astype(ml_dtypes.bfloat16)
    onesb = np.ones((128, 128), dtype=ml_dtypes.bfloat16)

    wq_bf = _to_bf16(wq)
    wk_bf = _to_bf16(wk)
    wv_bf = _to_bf16(wv)
    wo_bf = _to_bf16(wo)

    in_maps = []
    for core in range(N_CORES):
        b, r = core // TP, core % TP
        # o_proj rows for this core: its 1024 attn features
        wo_rows = wo_bf[r * HL * 128:(r + 1) * HL * 128, :]   # [1024, 4096]
        # per oc-panel lhsT tiles: [32, 128, HL*128]
        wop = wo_rows.reshape(HL, 128, H // 128, 128).transpose(2, 1, 0, 3)
        wop = np.ascontiguousarray(wop.reshape(H // 128, 128, HL * 128))
        in_maps.append({
            "xt": xts[b],
            "cos_t": cts[b],
            "sin_t": sin_eff[b],
            "wqp": _panelize(wq_bf[:, r * HL * 128:(r + 1) * HL * 128]),
            "wkp": _panelize(wk_bf[:, r * KVL * 128:(r + 1) * KVL * 128]),
            "wvp": _panelize(wv_bf[:, r * KVL * 128:(r + 1) * KVL * 128]),
            "wop": wop,
            "masks": masks,
            "onesb": onesb,
        })

    res = run_bass_kernel_spmd(nc, in_maps, core_ids=list(range(N_CORES)))
    LAST_RESULT = res

    out = np.zeros((B, S, H), dtype=np.float32)
    for core in range(N_CORES):
        b = core // TP
        out[b] += res.results[core]["out_t"].T
    return out
